# revision 1
# baseline (speedup 1.0000x reference)
"""Trainium2 Bass kernel for nn_AsymetricKernel (linear attention w/ InstanceNorm + 2D rotary).

Sharding: 8 cores = 4 batches x 2 head-groups (4 heads each). Fully independent
per core -- no collectives.

Per-core dataflow (all PE compute in bf16, fp32 PSUM accumulation):
  - q is projected twice, with Wq and a host-row-swapped Wq, giving q and
    swap(q) directly; rotary becomes two elementwise muls against transposed
    cos/sin tables and the rotate-half add is folded into the two accumulating
    u-matmuls (t1 = q*cos, t2sw = swap(q)*swap(sin)).
  - k, v are projected in natural [token, head*d] layout with per-head mean
    columns appended to the weights (mean comes out of the PE for free).
    Variance = sumsq/64 - mu^2 via one ACT square + one grouped DVE reduce.
  - InstanceNorm scales for k and v are combined (rc = rk*rv) into k; the
    v-mean term rides as an extra rhs column through the dots matmul and is
    folded back in the dots PSUM->SBUF copy (per-partition tensor_scalar).
  - dots = sum_n rot(k~) v~^T is built by two accumulating matmuls per head
    (cos part + swapped sin part, swap materialized by cheap strided copies),
    then laid out block-diagonally per head-pair so the u-matmuls run with
    K=128 at base_partition 0 (base-64 operands crash the HW).
  - u = (t1 + t2sw)^T-style: two K=128 matmuls per head-pair per 128-token
    chunk against the block-diag dots, accumulated in PSUM, copied out fp32.
"""

import numpy as np
import ml_dtypes

B, N, DIM, H, DH = 4, 8192, 512, 8, 64
HG = 2              # head groups (cores per batch) / head-pairs per core
HPG = H // HG       # heads per group = 4
E = HPG * DH        # 256 output cols per core
EPS = 1e-5
NT = 16             # n-tiles of 512
CPT = 4             # 128-chunks per n-tile
NCHUNK = NT * CPT   # 64
CC = DIM // 128     # 4 contraction chunks

_cache = {}


def _build_program():
    import concourse.tile as tile
    from concourse import bacc, mybir
    from contextlib import ExitStack

    f32 = mybir.dt.float32
    bf16 = mybir.dt.bfloat16

    # All inputs arrive in SBUF-native [128, free] layouts prepared on host
    # so every DMA is one contiguous read per partition.
    nc = bacc.Bacc(target_bir_lowering=False)
    uxT = nc.declare_dram_parameter("uxT", [NT, 128, CC * 512], bf16, isOutput=False)
    wq = nc.declare_dram_parameter("wq", [128, CC * E], bf16, isOutput=False)
    wqs = nc.declare_dram_parameter("wqs", [128, CC * E], bf16, isOutput=False)
    wkv = nc.declare_dram_parameter("wkv", [128, CC * 2 * E], bf16, isOutput=False)
    wm = nc.declare_dram_parameter("wm", [128, CC * 2 * HPG], bf16, isOutput=False)
    cosP = nc.declare_dram_parameter("cosP", [128, NCHUNK * DH], bf16, isOutput=False)
    sinN = nc.declare_dram_parameter("sinN", [128, NCHUNK * DH], bf16, isOutput=False)
    cosT = nc.declare_dram_parameter("cosT", [128, N], bf16, isOutput=False)
    sinTN = nc.declare_dram_parameter("sinTN", [128, N], bf16, isOutput=False)
    out = nc.declare_dram_parameter("out", [N, E], f32, isOutput=True)

    with ExitStack() as ctx:
        tc = ctx.enter_context(tile.TileContext(nc))
        consts = ctx.enter_context(tc.tile_pool(name="consts", bufs=1))
        store = ctx.enter_context(tc.tile_pool(name="store", bufs=1))

        # ---- persistent SBUF ----
        wq_sb = consts.tile([128, CC, E], bf16)
        wqs_sb = consts.tile([128, CC, E], bf16)
        wkv_sb = consts.tile([128, CC, 2 * E], bf16)
        wm_sb = consts.tile([128, CC, 2 * HPG], bf16)
        nc.sync.dma_start(wq_sb[:].rearrange("p c e -> p (c e)"), wq[:])
        nc.sync.dma_start(wqs_sb[:].rearrange("p c e -> p (c e)"), wqs[:])
        nc.sync.dma_start(wkv_sb[:].rearrange("p c e -> p (c e)"), wkv[:])
        nc.sync.dma_start(wm_sb[:].rearrange("p c e -> p (c e)"), wm[:])
        cosT_sb = consts.tile([128, N], bf16)
        sinTN_sb = consts.tile([128, N], bf16)
        nc.sync.dma_start(cosT_sb[:], cosT[:])
        nc.sync.dma_start(sinTN_sb[:], sinTN[:])
        cosP_sb = consts.tile([128, NCHUNK, DH], bf16)
        sinN_sb = consts.tile([128, NCHUNK, DH], bf16)
        nc.sync.dma_start(cosP_sb[:].rearrange("p t d -> p (t d)"), cosP[:])
        nc.sync.dma_start(sinN_sb[:].rearrange("p t d -> p (t d)"), sinN[:])

        t1T_sb = store.tile([128, HG, N], bf16)   # (q * cos)^T per head-pair
        t2T_sb = store.tile([128, HG, N], bf16)   # (swap(q) * swap(sin))^T
        # block-diagonal dots per head-pair (keeps u-matmul operands at
        # base_partition 0 with K=128)
        dotsA_sb = store.tile([128, HG, 128], bf16)

        with ExitStack() as p1:
            uxp = p1.enter_context(tc.tile_pool(name="uxp", bufs=3))
            work = p1.enter_context(tc.tile_pool(name="work", bufs=2))
            dwork = p1.enter_context(tc.tile_pool(name="dwork", bufs=3))
            stats = p1.enter_context(tc.tile_pool(name="stats", bufs=3))
            qps = p1.enter_context(tc.tile_pool(name="qps", bufs=3, space="PSUM"))
            kps = p1.enter_context(tc.tile_pool(name="kps", bufs=2, space="PSUM"))
            vps = p1.enter_context(tc.tile_pool(name="vps", bufs=2, space="PSUM"))
            dps = p1.enter_context(tc.tile_pool(name="dps", bufs=1, space="PSUM"))

            # paired dots: head-pair eb occupies partitions 0-63 (even head)
            # and 64-127 (odd head); cols 0-64 even / 65-129 odd (off-diag
            # blocks are junk we never read).
            dotsA = dps.tile([128, HG, 2 * (DH + 1)], f32)

            pend = {}

            def emit_dots(t):
                k1t, k2t, v5t = pend.pop(t)
                for ci in range(CPT):
                    gc = t * CPT + ci
                    for eb in range(HG):
                        lk1 = k1t[:, ci, 2 * eb:2 * eb + 2, :]
                        lk2 = k2t[:, ci, 2 * eb:2 * eb + 2, :]
                        vx = v5t[:, ci, 2 * eb:2 * eb + 2, :]
                        nc.tensor.matmul(
                            dotsA[:, eb, :], lk1, vx,
                            start=(gc == 0 and eb == 0), stop=False)
                        nc.tensor.matmul(
                            dotsA[:, eb, :], lk2, vx, start=False,
                            stop=(gc == NCHUNK - 1 and eb == HG - 1))

            for nt in range(NT):
                ns = nt * 512
                ux_t = uxp.tile([128, CC, 512], bf16)
                nc.sync.dma_start(ux_t[:].rearrange("p c n -> p (c n)"), uxT[nt, :, :])

                # ---- transposed q projections + rotary premul ----
                for eb in range(HG):
                    qp = qps.tile([128, 512], f32, tag="qp")
                    for cc in range(CC):
                        nc.tensor.matmul(
                            qp[:], wq_sb[:, cc, eb * 128:(eb + 1) * 128],
                            ux_t[:, cc, :],
                            start=(cc == 0), stop=(cc == CC - 1))
                    nc.vector.tensor_mul(
                        t1T_sb[:, eb, ns:ns + 512], qp[:], cosT_sb[:, ns:ns + 512])
                    qp2 = qps.tile([128, 512], f32, tag="qp")
                    for cc in range(CC):
                        nc.tensor.matmul(
                            qp2[:], wqs_sb[:, cc, eb * 128:(eb + 1) * 128],
                            ux_t[:, cc, :],
                            start=(cc == 0), stop=(cc == CC - 1))
                    nc.vector.tensor_mul(
                        t2T_sb[:, eb, ns:ns + 512], qp2[:], sinTN_sb[:, ns:ns + 512])

                # ---- k/v projections (merged, N=512) + mean columns ----
                # ksb5/v5 hold [chunk, head, 65] with the mean column in-line
                ksb5 = work.tile([128, CPT, HPG, DH + 1], bf16, tag="ksb5")
                v5 = dwork.tile([128, CPT, HPG, DH + 1], bf16, tag="v5")
                for ci in range(CPT):
                    kvp = kps.tile([128, 2 * E], f32, tag="kvp")
                    mp = vps.tile([128, 2 * HPG], f32, tag="mp")
                    for cc in range(CC):
                        nc.tensor.matmul(
                            kvp[:], ux_t[:, cc, ci * 128:(ci + 1) * 128],
                            wkv_sb[:, cc, :],
                            start=(cc == 0), stop=(cc == CC - 1))
                    for cc in range(CC):
                        nc.tensor.matmul(
                            mp[:], ux_t[:, cc, ci * 128:(ci + 1) * 128],
                            wm_sb[:, cc, :],
                            start=(cc == 0), stop=(cc == CC - 1))
                    nc.scalar.copy(ksb5[:, ci, :, 0:DH], kvp[:, 0:E].rearrange(
                        "p (g d) -> p g d", g=HPG))
                    nc.scalar.copy(v5[:, ci, :, 0:DH], kvp[:, E:2 * E].rearrange(
                        "p (g d) -> p g d", g=HPG))
                    nc.scalar.copy(ksb5[:, ci, :, DH:DH + 1],
                                   mp[:, 0:HPG].unsqueeze(-1))
                    nc.scalar.copy(v5[:, ci, :, DH:DH + 1],
                                   mp[:, HPG:2 * HPG].unsqueeze(-1))

                # ---- whole-tile stats: squares (ACT), grouped reduce, soup ----
                ksq = work.tile([128, CPT, HPG, DH], bf16, tag="ksq")
                vsq = work.tile([128, CPT, HPG, DH], bf16, tag="vsq")
                nc.scalar.square(ksq[:], ksb5[:, :, :, 0:DH])
                nc.scalar.square(vsq[:], v5[:, :, :, 0:DH])
                ss = stats.tile([128, 2, CPT, HPG], f32, tag="ss")
                nc.vector.tensor_reduce(
                    out=ss[:, 0], in_=ksq[:], axis=mybir.AxisListType.X,
                    op=mybir.AluOpType.add)
                nc.vector.tensor_reduce(
                    out=ss[:, 1], in_=vsq[:], axis=mybir.AxisListType.X,
                    op=mybir.AluOpType.add)
                SW = 2 * CPT * HPG
                tmp = stats.tile([128, SW], f32, tag="tmp")
                tmp2 = stats.tile([128, SW], f32, tag="tmp2")
                rr = stats.tile([128, 2, CPT, HPG], f32, tag="rr")
                mu2 = stats.tile([128, 2, CPT, HPG], f32, tag="mu2")
                nc.vector.tensor_mul(
                    mu2[:, 0], ksb5[:, :, :, DH], ksb5[:, :, :, DH])
                nc.vector.tensor_mul(
                    mu2[:, 1], v5[:, :, :, DH], v5[:, :, :, DH])
                nc.vector.tensor_scalar(
                    out=tmp[:], in0=ss[:].rearrange("p a c g -> p (a c g)"),
                    scalar1=1.0 / DH, scalar2=EPS,
                    op0=mybir.AluOpType.mult, op1=mybir.AluOpType.add)
                nc.vector.tensor_sub(
                    tmp[:], tmp[:], mu2[:].rearrange("p a c g -> p (a c g)"))
                nc.scalar.sqrt(tmp[:], tmp[:])
                nc.vector.reciprocal(rr[:].rearrange("p a c g -> p (a c g)"), tmp[:])
                rcs = stats.tile([128, CPT, HPG], f32, tag="rcs")
                mcs = stats.tile([128, CPT, HPG], f32, tag="mcs")
                nc.vector.tensor_mul(rcs[:], rr[:, 0], rr[:, 1])
                nc.vector.tensor_mul(mcs[:], ksb5[:, :, :, DH], rcs[:])
                nc.vector.tensor_scalar(
                    out=mcs[:], in0=mcs[:], scalar1=-1.0, scalar2=None,
                    op0=mybir.AluOpType.mult)

                # ---- whole-tile normalize + rotary parts (gpsimd/DVE) ----
                ktn = work.tile([128, CPT, HPG, DH], bf16, tag="ktn")
                nc.gpsimd.tensor_mul(
                    ktn[:], ksb5[:, :, :, 0:DH],
                    rcs[:].unsqueeze(-1).broadcast_to([128, CPT, HPG, DH]))
                nc.gpsimd.tensor_add(
                    ktn[:], ktn[:],
                    mcs[:].unsqueeze(-1).broadcast_to([128, CPT, HPG, DH]))
                ktns = work.tile([128, CPT, HPG, DH], bf16, tag="ktns")
                kv_ = ktn[:].rearrange("p c g (b h s) -> p (c g b) h s", b=2, s=16)
                ks_ = ktns[:].rearrange("p c g (b h s) -> p (c g b) h s", b=2, s=16)
                nc.vector.tensor_copy(ks_[:, :, 0:1, :], kv_[:, :, 1:2, :])
                nc.vector.tensor_copy(ks_[:, :, 1:2, :], kv_[:, :, 0:1, :])
                k1 = dwork.tile([128, CPT, HPG, DH], bf16, tag="k1")
                k2sw = dwork.tile([128, CPT, HPG, DH], bf16, tag="k2sw")
                tslice = slice(nt * CPT, (nt + 1) * CPT)
                nc.vector.tensor_mul(
                    k1[:], ktn[:],
                    cosP_sb[:, tslice, :].unsqueeze(2).broadcast_to(
                        [128, CPT, HPG, DH]))
                nc.gpsimd.tensor_mul(
                    k2sw[:], ktns[:],
                    sinN_sb[:, tslice, :].unsqueeze(2).broadcast_to(
                        [128, CPT, HPG, DH]))

                # ---- dots accumulation for the PREVIOUS tile ----
                # (software pipeline: PE never waits on this tile's
                # elementwise chain; it runs the next tile's projections
                # first, then the previous tile's dots)
                if nt > 0:
                    emit_dots(nt - 1)
                pend[nt] = (k1, k2sw, v5)
            emit_dots(NT - 1)

            # ---- finalize dots: add g col, scale 1/N, cast bf16, blockdiag ----
            nc.vector.memset(dotsA_sb[:], 0.0)
            for eb in range(HG):
                gA = stats.tile([128, 1], f32, tag="gA")
                nc.scalar.copy(gA[0:64, :], dotsA[0:64, eb, DH:DH + 1])
                nc.scalar.copy(gA[64:128, :], dotsA[64:128, eb, 2 * DH + 1:2 * DH + 2])
                nc.vector.tensor_scalar(
                    out=dotsA_sb[0:64, eb, 0:DH], in0=dotsA[0:64, eb, 0:DH],
                    scalar1=gA[0:64, :], scalar2=1.0 / N,
                    op0=mybir.AluOpType.add, op1=mybir.AluOpType.mult)
                nc.vector.tensor_scalar(
                    out=dotsA_sb[64:128, eb, DH:2 * DH],
                    in0=dotsA[64:128, eb, DH + 1:2 * DH + 1],
                    scalar1=gA[64:128, :], scalar2=1.0 / N,
                    op0=mybir.AluOpType.add, op1=mybir.AluOpType.mult)

        # ---- pass 2: u = t1T^T @ dots + t2T^T @ dots ----
        with ExitStack() as p2:
            ups = p2.enter_context(tc.tile_pool(name="ups", bufs=4, space="PSUM"))
            uout = p2.enter_context(tc.tile_pool(name="uout", bufs=4))
            for gc in range(NCHUNK):
                up = ups.tile([128, E], f32)
                for eb in range(HG):
                    nsl = slice(gc * 128, (gc + 1) * 128)
                    nc.tensor.matmul(up[:, eb * 128:(eb + 1) * 128],
                                     t1T_sb[:, eb, nsl], dotsA_sb[:, eb, :],
                                     start=(eb == 0), stop=False)
                    nc.tensor.matmul(up[:, eb * 128:(eb + 1) * 128],
                                     t2T_sb[:, eb, nsl], dotsA_sb[:, eb, :],
                                     start=False, stop=(eb == HG - 1))
                u_sb = uout.tile([128, E], f32)
                if gc % 2 == 0:
                    nc.vector.tensor_copy(u_sb[:], up[:])
                else:
                    nc.scalar.copy(u_sb[:], up[:])
                nc.sync.dma_start(out[gc * 128:(gc + 1) * 128, :], u_sb[:])

    nc.finalize()
    return nc


def _swap_cols(a):
    """Swap 16-halves within 32-blocks along the last axis (len 64)."""
    s = a.reshape(*a.shape[:-1], 2, 2, 16)
    return np.concatenate([s[..., 1:2, :], s[..., 0:1, :]], axis=-2).reshape(a.shape)


def _host_prep(u_x, pos_x, Wq, Wk, Wv):
    bf = ml_dtypes.bfloat16
    invf = 1.0 / 10000.0 ** (np.arange(0, 32, 2, dtype=np.float64) / 32)
    t64 = pos_x[0].astype(np.float64) * 64.0
    fx = t64[:, 0:1] * invf[None, :]
    fy = t64[:, 1:2] * invf[None, :]
    cx, sx = np.cos(fx), np.sin(fx)
    cy, sy = np.cos(fy), np.sin(fy)
    cosPf = np.concatenate([cx, cx, cy, cy], 1).astype(np.float32)    # [N, 64]
    sinAf = np.concatenate([sx, -sx, sy, -sy], 1).astype(np.float32)  # [N, 64]
    sinNf = -sinAf

    def chunked(t):  # [N, 64] -> [128, NCHUNK*64], partition = token % 128
        return np.ascontiguousarray(
            t.reshape(NCHUNK, 128, DH).transpose(1, 0, 2).reshape(128, -1)).astype(bf)

    cosP = chunked(cosPf)
    sinN = chunked(sinNf)
    cosT = np.ascontiguousarray(np.tile(cosPf.T, (2, 1))).astype(bf)   # [128, N]
    sinTN = np.ascontiguousarray(np.tile(sinNf.T, (2, 1))).astype(bf)

    def wlayout(wT):  # [512, E'] -> [128, CC*E'] partition-native
        Ep = wT.shape[1]
        return np.ascontiguousarray(
            wT.reshape(CC, 128, Ep).transpose(1, 0, 2).reshape(128, -1)).astype(bf)

    in_maps = []
    for b in range(B):
        # [N, DIM] -> [NT, 128, CC*512]: uxt[nt, p, cc*512+j] = u_x[nt*512+j, cc*128+p]
        uxtb = np.ascontiguousarray(
            u_x[b].reshape(NT, 512, CC, 128).transpose(0, 3, 2, 1).reshape(
                NT, 128, CC * 512)).astype(bf)
        for hg in range(HG):
            sl = slice(hg * E, (hg + 1) * E)
            wq_g = Wq[sl]                          # [256, 512]
            # row-swapped Wq: swap within each head's 64 rows
            wq_sw = wq_g.reshape(HPG, DH, DIM)
            wq_sw = _swap_cols(wq_sw.transpose(0, 2, 1)).transpose(0, 2, 1).reshape(
                E, DIM)
            wbar_k = Wk[sl].reshape(HPG, DH, DIM).mean(1).T            # [512, 4]
            wbar_vn = -Wv[sl].reshape(HPG, DH, DIM).mean(1).T
            in_maps.append({
                "uxT": uxtb,
                "wq": wlayout(wq_g.T.astype(np.float32)),
                "wqs": wlayout(wq_sw.T.astype(np.float32)),
                "wkv": wlayout(np.concatenate([Wk[sl].T, Wv[sl].T], 1)),
                "wm": wlayout(np.concatenate([wbar_k, wbar_vn], 1)),
                "cosP": cosP, "sinN": sinN, "cosT": cosT, "sinTN": sinTN,
            })
    return in_maps


def kernel(u_x, pos_x, Wq, Wk, Wv, _trace=False, _trace_dir=None):
    from concourse.bass_utils import run_bass_kernel_spmd

    if "nc" not in _cache:
        _cache["nc"] = _build_program()
    nc = _cache["nc"]

    in_maps = _host_prep(
        np.asarray(u_x, np.float32), np.asarray(pos_x, np.float32),
        np.asarray(Wq, np.float32), np.asarray(Wk, np.float32),
        np.asarray(Wv, np.float32))

    kw = {}
    if _trace:
        kw = {"trace": True, "tmpdir": _trace_dir}
    res = run_bass_kernel_spmd(nc, in_maps, core_ids=list(range(8)), **kw)
    _cache["last_result"] = res

    out = np.empty((B, N, H * DH), np.float32)
    for i in range(8):
        b, hg = divmod(i, HG)
        out[b, :, hg * E:(hg + 1) * E] = res.results[i]["out"]
    return out



# revision 35
# speedup vs baseline: 1.5201x; 1.5201x over previous
"""Trainium2 Bass kernel for nn_AsymetricKernel (linear attention w/ InstanceNorm + 2D rotary).

Sharding: 8 cores = 4 batches x 2 head-groups (4 heads each). Fully independent
per core -- no collectives.

v3 design notes (all PE compute bf16, fp32 PSUM accumulation):
  - InstanceNorm mean subtraction is linear in the projection weights, so the
    host centers Wk/Wv per head (W~ = (I - 11^T/64) W). Variance is then just
    sumsq/64: one ACT square + one DVE grouped reduce + fused
    sqrt(ss/64+eps) on ACT; rc = rsqrt(vk)*rsqrt(vv) is folded into k only.
  - rotate-half swaps are ELIMINATED by reindexing the contraction (sigma is
    an involution): for q, t2 = q * sinTsw (host-swapped rows of the sin
    table) contracted against a row-permuted dots (dotsS); for k, the sin
    matmuls use a host-d-permuted sin table and accumulate into a second
    PSUM region G, and the finalize computes dots[c] = D1[c] + G[sigma(c)]
    (and dotsS[c] = D1[sigma(c)] + G[c]) with small 16-partition-block adds.
  - 1/N is folded into the host cos/sin tables of the q path, so the dots
    finalize is pure adds + bf16 casts.
  - pass 2 uses dots/dotsS as stationary operands and streams 512 tokens per
    matmul, producing u transposed ([vfeat, tok]); output staged in SBUF and
    written bf16 in 2048-token batches (8 DMA dispatches); host undoes the
    transpose. DVE tensor_copy with f32->bf16 conversion is avoided (broken
    rounding) -- tensor_scalar mult-1.0 is used instead.
"""

import numpy as np
import ml_dtypes

B, N, DIM, H, DH = 4, 8192, 512, 8, 64
HG = 2              # head groups (cores per batch) / head-pairs per core
HPG = H // HG       # heads per group = 4
E = HPG * DH        # 256 output cols per core
EPS = 1e-5
NT = 16             # n-tiles of 512
CPT = 4             # 128-chunks per n-tile
NCHUNK = NT * CPT   # 64
CC = DIM // 128     # 4 contraction chunks
OB = 8              # output staging: n-tiles per DMA batch

_cache = {}


def _build_program():
    import concourse.tile as tile
    from concourse import bacc, mybir
    from contextlib import ExitStack

    f32 = mybir.dt.float32
    bf16 = mybir.dt.bfloat16

    nc = bacc.Bacc(target_bir_lowering=False)
    uxT = nc.declare_dram_parameter("uxT", [NT, 128, (CC + 2) * 512], bf16, isOutput=False)
    wq = nc.declare_dram_parameter("wq", [128, CC * E], bf16, isOutput=False)
    wkv = nc.declare_dram_parameter("wkv", [128, CC * 2 * E], bf16, isOutput=False)
    cosP = nc.declare_dram_parameter("cosP", [128, NCHUNK * DH], bf16, isOutput=False)
    sinNs = nc.declare_dram_parameter("sinNs", [128, NCHUNK * DH], bf16, isOutput=False)
    outT = nc.declare_dram_parameter("outT", [HG, 128, N], bf16, isOutput=True)

    with ExitStack() as ctx:
        tc = ctx.enter_context(tile.TileContext(nc))
        consts = ctx.enter_context(tc.tile_pool(name="consts", bufs=1))
        store = ctx.enter_context(tc.tile_pool(name="store", bufs=1))

        # ---- persistent SBUF (weights first: first matmuls need only these) ----
        wq_sb = consts.tile([128, CC, E], bf16)
        wkv_sb = consts.tile([128, CC, 2 * E], bf16)
        nc.sync.dma_start(wq_sb[:].rearrange("p c e -> p (c e)"), wq[:])
        cosP_sb = consts.tile([128, NCHUNK, DH], bf16)
        sinNs_sb = consts.tile([128, NCHUNK, DH], bf16)

        eps_sb = consts.tile([128, 1], f32)
        nc.vector.memset(eps_sb[:], EPS)

        # t12T[a]: a=0 -> (q * cos/N)^T, a=1 -> (q * sinTsw/N)^T
        t12T_sb = store.tile([128, 2, HG, N], bf16)
        dotsA_sb = store.tile([128, HG, 128], bf16)  # block-diag D1+Sg(G)
        dotsS_sb = store.tile([128, HG, 128], bf16)  # sigma-row-permuted dots

        with ExitStack() as p1:
            uxp = p1.enter_context(tc.tile_pool(name="uxp", bufs=4))
            work = p1.enter_context(tc.tile_pool(name="work", bufs=3))
            dwork = p1.enter_context(tc.tile_pool(name="dwork", bufs=4))
            stats = p1.enter_context(tc.tile_pool(name="stats", bufs=3))
            qps = p1.enter_context(tc.tile_pool(name="qps", bufs=3, space="PSUM"))
            kps = p1.enter_context(tc.tile_pool(name="kps", bufs=4, space="PSUM"))
            dps = p1.enter_context(tc.tile_pool(name="dps", bufs=1, space="PSUM"))

            # dots PSUM: [which(D1/G), eb, 128] -- one 2KB bank, 4 accumulation
            # regions. Head-pair eb: partitions 0-63 even head, 64-127 odd;
            # off-diagonal 64x64 blocks are junk.
            dots = dps.tile([128, 2, HG, 128], f32)

            pend = {}

            # NOTE: start=True zeroes the WHOLE PSUM bank, not just the
            # matmul's own region -- so only the very first matmul into the
            # bank may carry start=True; all four regions then accumulate
            # onto the bank-wide zero.
            def emit_dots(t):
                k1t, k2t, v5t = pend.pop(t)
                for ci in range(CPT):
                    gc = t * CPT + ci
                    last = gc == NCHUNK - 1
                    for eb in range(HG):
                        vx = v5t[:, 1, ci, 2 * eb:2 * eb + 2, :]
                        nc.tensor.matmul(
                            dots[:, 0, eb, :], k1t[:, ci, 2 * eb:2 * eb + 2, :],
                            vx, start=(gc == 0 and eb == 0), stop=last)
                        nc.tensor.matmul(
                            dots[:, 1, eb, :], k2t[:, ci, 2 * eb:2 * eb + 2, :],
                            vx, start=False, stop=last)

            for nt in range(NT):
                ns = nt * 512
                ux_t = uxp.tile([128, CC + 2, 512], bf16)
                nc.sync.dma_start(ux_t[:].rearrange("p c n -> p (c n)"), uxT[nt, :, :])
                if nt == 0:
                    # big tables land after tile-0's inputs so the first
                    # matmuls aren't stuck behind 2.7MB of table DMA
                    nc.sync.dma_start(
                        wkv_sb[:].rearrange("p c e -> p (c e)"), wkv[:])
                    nc.sync.dma_start(
                        cosP_sb[:].rearrange("p t d -> p (t d)"), cosP[:])
                    nc.sync.dma_start(
                        sinNs_sb[:].rearrange("p t d -> p (t d)"), sinNs[:])

                # ---- k/v projections; kv5 = [a(k/v), ci, g, d] ----
                kv5 = work.tile([128, 2, CPT, HPG, DH], bf16, tag="kv5")
                for ci in range(CPT):
                    kvp = kps.tile([128, 2 * E], f32, tag="kvp")
                    for cc in range(CC):
                        nc.tensor.matmul(
                            kvp[:], ux_t[:, cc, ci * 128:(ci + 1) * 128],
                            wkv_sb[:, cc, :],
                            start=(cc == 0), stop=(cc == CC - 1))
                    nc.scalar.copy(
                        kv5[:, :, ci, :, :],
                        kvp[:].rearrange("p (a g d) -> p a g d", a=2, g=HPG))

                # ---- stats: square, grouped reduce, fused sqrt(ss/64+eps) ----
                sq5 = work.tile([128, 2, CPT, HPG, DH], bf16, tag="sq5")
                nc.scalar.square(sq5[:], kv5[:])
                ss = stats.tile([128, 2, CPT, HPG], f32, tag="ss")
                nc.vector.tensor_reduce(
                    out=ss[:], in_=sq5[:], axis=mybir.AxisListType.X,
                    op=mybir.AluOpType.add)
                ts = stats.tile([128, 2, CPT * HPG], f32, tag="ts")
                nc.scalar.activation(
                    ts[:].rearrange("p a x -> p (a x)"),
                    ss[:].rearrange("p a c g -> p (a c g)"),
                    mybir.ActivationFunctionType.Sqrt,
                    bias=eps_sb[:], scale=1.0 / DH)
                sd = stats.tile([128, CPT * HPG], f32, tag="sd")
                nc.vector.tensor_mul(sd[:], ts[:, 0], ts[:, 1])
                rc = stats.tile([128, CPT, HPG], f32, tag="rc")
                nc.vector.reciprocal(rc[:].rearrange("p c g -> p (c g)"), sd[:])

                # ---- normalize k (rc folded) + rotary muls (no swap) ----
                ktn = dwork.tile([128, CPT, HPG, DH], bf16, tag="ktn")
                if nt < NT - 2:
                    nc.gpsimd.tensor_mul(
                        ktn[:], kv5[:, 0],
                        rc[:].unsqueeze(-1).broadcast_to([128, CPT, HPG, DH]))
                else:
                    nc.gpsimd.tensor_mul(
                        ktn[:, 0:2], kv5[:, 0, 0:2],
                        rc[:, 0:2].unsqueeze(-1).broadcast_to([128, 2, HPG, DH]))
                    nc.vector.tensor_mul(
                        ktn[:, 2:4], kv5[:, 0, 2:4],
                        rc[:, 2:4].unsqueeze(-1).broadcast_to([128, 2, HPG, DH]))
                k1 = dwork.tile([128, CPT, HPG, DH], bf16, tag="k1")
                k2n = dwork.tile([128, CPT, HPG, DH], bf16, tag="k2n")
                tsl = slice(nt * CPT, (nt + 1) * CPT)
                nc.vector.tensor_mul(
                    k1[:, 0:2], ktn[:, 0:2],
                    cosP_sb[:, nt * CPT:nt * CPT + 2, :].unsqueeze(2)
                    .broadcast_to([128, 2, HPG, DH]))
                nc.gpsimd.tensor_mul(
                    k1[:, 2:4], ktn[:, 2:4],
                    cosP_sb[:, nt * CPT + 2:nt * CPT + 4, :].unsqueeze(2)
                    .broadcast_to([128, 2, HPG, DH]))
                if nt < NT - 2:
                    nc.gpsimd.tensor_mul(
                        k2n[:], ktn[:],
                        sinNs_sb[:, tsl, :].unsqueeze(2).broadcast_to(
                            [128, CPT, HPG, DH]))
                else:
                    nc.gpsimd.tensor_mul(
                        k2n[:, 0:2], ktn[:, 0:2],
                        sinNs_sb[:, nt * CPT:nt * CPT + 2, :].unsqueeze(2)
                        .broadcast_to([128, 2, HPG, DH]))
                    nc.vector.tensor_mul(
                        k2n[:, 2:4], ktn[:, 2:4],
                        sinNs_sb[:, nt * CPT + 2:nt * CPT + 4, :].unsqueeze(2)
                        .broadcast_to([128, 2, HPG, DH]))

                # ---- transposed q projection + rotary premul (swap-free) ----
                for eb in range(HG):
                    qp = qps.tile([128, 512], f32, tag="qp")
                    for cc in range(CC):
                        nc.tensor.matmul(
                            qp[:], wq_sb[:, cc, eb * 128:(eb + 1) * 128],
                            ux_t[:, cc, :],
                            start=(cc == 0), stop=(cc == CC - 1))
                    nc.vector.tensor_mul(
                        t12T_sb[:, :, eb, ns:ns + 512],
                        qp[:].unsqueeze(1).broadcast_to([128, 2, 512]),
                        ux_t[:, CC:CC + 2, :])

                # ---- dots, two tiles behind (software pipeline) ----
                if nt > 1:
                    emit_dots(nt - 2)
                pend[nt] = (k1, k2n, kv5)
            emit_dots(NT - 2)
            emit_dots(NT - 1)

            # ---- finalize: dots[c] = D1[c] + G[sigma(c)] (diag blocks only),
            #      dotsS[c] = D1[sigma(c)] + G[c]; bf16, no scaling (1/N is in
            #      the q tables). sigma swaps 16-blocks within 32-blocks.
            nc.vector.memset(dotsA_sb[:], 0.0)
            nc.gpsimd.memset(dotsS_sb[:], 0.0)
            # gpsimd can't read PSUM, and two-SBUF-input TensorTensor needs
            # equal partition bases -- but PSUM+SBUF with shifted bases is
            # allowed. Stage G in SBUF, then all adds on vector.
            gsb = stats.tile([128, HG, 128], f32, tag="gsb")
            nc.scalar.copy(gsb[:], dots[:, 1, :, :])
            for eb in range(HG):
                for hh in range(2):          # head-in-pair: rows/cols 64-block
                    r0, c0 = hh * 64, hh * 64
                    for b in range(2):       # 32-partition sigma blocks
                        rb = r0 + b * 32
                        rs = r0 + (b ^ 1) * 32   # sigma partner block
                        cols = slice(c0, c0 + 64)
                        nc.vector.tensor_add(
                            dotsA_sb[rb:rb + 32, eb, cols],
                            dots[rb:rb + 32, 0, eb, cols],
                            gsb[rs:rs + 32, eb, cols])
                        nc.vector.tensor_add(
                            dotsS_sb[rb:rb + 32, eb, cols],
                            dots[rs:rs + 32, 0, eb, cols],
                            gsb[rb:rb + 32, eb, cols])

        # ---- pass 2: uT[vfeat, tok] = dotsA^T@t1T + dotsS^T@t2T ----
        with ExitStack() as p2:
            ups = p2.enter_context(tc.tile_pool(name="ups", bufs=6, space="PSUM"))
            uout = p2.enter_context(tc.tile_pool(name="uout", bufs=2))
            ostage = [None, None]
            for nt in range(NT):
                ns = nt * 512
                if nt % OB == 0:
                    ostage = [uout.tile([128, OB, 512], bf16, tag=f"os{eb}",
                                        name=f"os{eb}") for eb in range(HG)]
                for eb in range(HG):
                    up = ups.tile([128, 512], f32)
                    nc.tensor.matmul(up[:], dotsA_sb[:, eb, :],
                                     t12T_sb[:, 0, eb, ns:ns + 512],
                                     start=True, stop=False)
                    nc.tensor.matmul(up[:], dotsS_sb[:, eb, :],
                                     t12T_sb[:, 1, eb, ns:ns + 512],
                                     start=False, stop=True)
                    dst = ostage[eb][:, nt % OB, :]
                    if (nt + eb) % 2 == 0:
                        nc.vector.tensor_scalar(
                            out=dst, in0=up[:], scalar1=1.0, scalar2=None,
                            op0=mybir.AluOpType.mult)
                    else:
                        nc.scalar.copy(dst, up[:])
                if nt % OB == OB - 1:
                    bs = (nt // OB) * OB * 512
                    for eb in range(HG):
                        nc.sync.dma_start(
                            outT[eb, :, bs:bs + OB * 512],
                            ostage[eb][:].rearrange("p o n -> p (o n)"))

    nc.finalize()
    return nc


def _center_heads(w):
    """InstanceNorm mean-subtraction folded into weights: per 64-row head
    block, subtract the block's column means."""
    wh = w.reshape(HPG, DH, DIM)
    return (wh - wh.mean(axis=1, keepdims=True)).reshape(E, DIM)


# rotary-axis permutation: pairs (c, sigma(c)) land 32 apart, so the sigma
# reindex in the kernel is a clean 32-partition-block swap
_PI = np.concatenate([np.arange(0, 16), np.arange(32, 48),
                      np.arange(16, 32), np.arange(48, 64)])


def _permq(w):
    """Permute per-head rotary output rows of a [E, DIM] projection weight."""
    return w.reshape(HPG, DH, DIM)[:, _PI, :].reshape(E, DIM)


def _host_prep(u_x, pos_x, Wq, Wk, Wv):
    bf = ml_dtypes.bfloat16
    invf = 1.0 / 10000.0 ** (np.arange(0, 32, dtype=np.float64)[::2] / 32)
    t64 = pos_x[0].astype(np.float64) * 64.0
    fx = t64[:, 0:1] * invf[None, :]
    fy = t64[:, 1:2] * invf[None, :]
    cx, sx = np.cos(fx), np.sin(fx)
    cy, sy = np.cos(fy), np.sin(fy)
    cosPf = np.concatenate([cx, cx, cy, cy], 1).astype(np.float32)[:, _PI]
    sinNf = -np.concatenate([sx, -sx, sy, -sy], 1).astype(np.float32)[:, _PI]

    sig64 = np.arange(64) ^ 32
    sinNsf = sinNf[:, sig64]          # d-permuted for the k-side G matmuls

    def chunked(t):  # [N, 64] -> [128, NCHUNK*64], partition = token % 128
        return np.ascontiguousarray(
            t.reshape(NCHUNK, 128, DH).transpose(1, 0, 2).reshape(128, -1)).astype(bf)

    cosP = chunked(cosPf)
    sinNs = chunked(sinNsf)

    # transposed q-path tables with 1/N folded in; sin rows sigma-permuted
    cosT = np.tile(cosPf.T, (2, 1)) / N            # [128, N]
    sinTsw = np.tile(sinNsf.T, (2, 1)) / N
    rot2c = np.ascontiguousarray(
        np.stack([cosT.reshape(128, NT, 512), sinTsw.reshape(128, NT, 512)],
                 axis=2).transpose(1, 0, 2, 3).reshape(NT, 128, 1024)).astype(bf)
    rot2f = rot2c.astype(np.float32)               # appended to uxT per tile

    def wlayout(wT):  # [512, E'] -> [128, CC*E'] partition-native
        Ep = wT.shape[1]
        return np.ascontiguousarray(
            wT.reshape(CC, 128, Ep).transpose(1, 0, 2).reshape(128, -1)).astype(bf)

    in_maps = []
    for b in range(B):
        uxtb = np.ascontiguousarray(np.concatenate([
            u_x[b].reshape(NT, 512, CC, 128).transpose(0, 3, 2, 1).reshape(
                NT, 128, CC * 512),
            rot2f], axis=2)).astype(bf)
        for hg in range(HG):
            sl = slice(hg * E, (hg + 1) * E)
            wk_c = _permq(_center_heads(Wk[sl]))
            wv_c = _center_heads(Wv[sl])
            in_maps.append({
                "uxT": uxtb,
                "wq": wlayout(_permq(Wq[sl]).T.astype(np.float32)),
                "wkv": wlayout(np.concatenate([wk_c.T, wv_c.T], 1)),
                "cosP": cosP, "sinNs": sinNs,
            })
    return in_maps


def kernel(u_x, pos_x, Wq, Wk, Wv, _trace=False, _trace_dir=None):
    from concourse.bass_utils import run_bass_kernel_spmd

    if "nc" not in _cache:
        _cache["nc"] = _build_program()
    nc = _cache["nc"]

    in_maps = _host_prep(
        np.asarray(u_x, np.float32), np.asarray(pos_x, np.float32),
        np.asarray(Wq, np.float32), np.asarray(Wk, np.float32),
        np.asarray(Wv, np.float32))

    kw = {}
    if _trace:
        kw = {"trace": True, "tmpdir": _trace_dir}
    res = run_bass_kernel_spmd(nc, in_maps, core_ids=list(range(8)), **kw)
    _cache["last_result"] = res

    out = np.empty((B, N, H * DH), np.float32)
    for i in range(8):
        b, hg = divmod(i, HG)
        oT = res.results[i]["outT"].astype(np.float32)   # [HG, 128, N]
        out[b, :, hg * E:(hg + 1) * E] = oT.reshape(HG * 128, N).T
    return out


# revision 36
# speedup vs baseline: 1.8268x; 1.2018x over previous
"""Trainium2 Bass kernel for nn_AsymetricKernel (linear attention w/ InstanceNorm + 2D rotary).

Sharding: 8 cores = 4 batches x 2 head-groups (4 heads each). Fully independent
per core -- no collectives.

v3 design notes (all PE compute bf16, fp32 PSUM accumulation):
  - InstanceNorm mean subtraction is linear in the projection weights, so the
    host centers Wk/Wv per head (W~ = (I - 11^T/64) W). Variance is then just
    sumsq/64: one ACT square + one DVE grouped reduce + fused
    sqrt(ss/64+eps) on ACT; rc = rsqrt(vk)*rsqrt(vv) is folded into k only.
  - rotate-half swaps are ELIMINATED by reindexing the contraction (sigma is
    an involution): for q, t2 = q * sinTsw (host-swapped rows of the sin
    table) contracted against a row-permuted dots (dotsS); for k, the sin
    matmuls use a host-d-permuted sin table and accumulate into a second
    PSUM region G, and the finalize computes dots[c] = D1[c] + G[sigma(c)]
    (and dotsS[c] = D1[sigma(c)] + G[c]) with small 16-partition-block adds.
  - 1/N is folded into the host cos/sin tables of the q path, so the dots
    finalize is pure adds + bf16 casts.
  - pass 2 uses dots/dotsS as stationary operands and streams 512 tokens per
    matmul, producing u transposed ([vfeat, tok]); output staged in SBUF and
    written bf16 in 2048-token batches (8 DMA dispatches); host undoes the
    transpose. DVE tensor_copy with f32->bf16 conversion is avoided (broken
    rounding) -- tensor_scalar mult-1.0 is used instead.
"""

import numpy as np
import ml_dtypes

B, N, DIM, H, DH = 4, 8192, 512, 8, 64
HG = 2              # head groups (cores per batch) / head-pairs per core
HPG = H // HG       # heads per group = 4
E = HPG * DH        # 256 output cols per core
EPS = 1e-5
NT = 16             # n-tiles of 512
CPT = 4             # 128-chunks per n-tile
NCHUNK = NT * CPT   # 64
CC = DIM // 128     # 4 contraction chunks
OB = 8              # output staging: n-tiles per DMA batch

_cache = {}


def _build_program():
    import concourse.tile as tile
    from concourse import bacc, mybir
    from contextlib import ExitStack

    f32 = mybir.dt.float32
    bf16 = mybir.dt.bfloat16

    nc = bacc.Bacc(target_bir_lowering=False)
    uxT = nc.declare_dram_parameter("uxT", [NT, 128, (CC + 2) * 512], bf16, isOutput=False)
    wq = nc.declare_dram_parameter("wq", [128, CC * E], bf16, isOutput=False)
    wkv = nc.declare_dram_parameter("wkv", [128, CC * 2 * E], bf16, isOutput=False)
    cosP = nc.declare_dram_parameter("cosP", [128, NCHUNK * DH], bf16, isOutput=False)
    sinNs = nc.declare_dram_parameter("sinNs", [128, NCHUNK * DH], bf16, isOutput=False)
    outT = nc.declare_dram_parameter("outT", [HG, 128, N], bf16, isOutput=True)

    with ExitStack() as ctx:
        tc = ctx.enter_context(tile.TileContext(nc))
        consts = ctx.enter_context(tc.tile_pool(name="consts", bufs=1))
        store = ctx.enter_context(tc.tile_pool(name="store", bufs=1))

        # ---- persistent SBUF (weights first: first matmuls need only these) ----
        wq_sb = consts.tile([128, CC, E], bf16)
        wkv_sb = consts.tile([128, CC, 2 * E], bf16)
        nc.sync.dma_start(wq_sb[:].rearrange("p c e -> p (c e)"), wq[:])
        cosP_sb = consts.tile([128, NCHUNK, DH], bf16)
        sinNs_sb = consts.tile([128, NCHUNK, DH], bf16)

        eps_sb = consts.tile([128, 1], f32)
        nc.vector.memset(eps_sb[:], EPS)

        # t12T[a]: a=0 -> (q * cos/N)^T, a=1 -> (q * sinTsw/N)^T
        t12T_sb = store.tile([128, 2, HG, N], bf16)
        dotsA_sb = store.tile([128, HG, 128], bf16)  # block-diag D1+Sg(G)
        dotsS_sb = store.tile([128, HG, 128], bf16)  # sigma-row-permuted dots

        with ExitStack() as p1:
            uxp = p1.enter_context(tc.tile_pool(name="uxp", bufs=4))
            work = p1.enter_context(tc.tile_pool(name="work", bufs=3))
            dwork = p1.enter_context(tc.tile_pool(name="dwork", bufs=4))
            stats = p1.enter_context(tc.tile_pool(name="stats", bufs=3))
            qps = p1.enter_context(tc.tile_pool(name="qps", bufs=3, space="PSUM"))
            kps = p1.enter_context(tc.tile_pool(name="kps", bufs=4, space="PSUM"))
            dps = p1.enter_context(tc.tile_pool(name="dps", bufs=1, space="PSUM"))

            # dots PSUM: [which(D1/G), eb, 128] -- one 2KB bank, 4 accumulation
            # regions. Head-pair eb: partitions 0-63 even head, 64-127 odd;
            # off-diagonal 64x64 blocks are junk.
            dots = dps.tile([128, 2, HG, 128], f32)

            pend = {}

            # NOTE: start=True zeroes the WHOLE PSUM bank, not just the
            # matmul's own region -- so only the very first matmul into the
            # bank may carry start=True; all four regions then accumulate
            # onto the bank-wide zero.
            def emit_dots(t):
                k1t, k2t, v5t = pend.pop(t)
                order = ([(ci, eb) for ci in range(CPT) for eb in range(HG)]
                         if t < NT - 1 else
                         [(ci, eb) for eb in range(HG) for ci in range(CPT)])
                for ci, eb in order:
                    gc = t * CPT + ci
                    last = gc == NCHUNK - 1
                    vx = v5t[:, 1, ci, 2 * eb:2 * eb + 2, :]
                    nc.tensor.matmul(
                        dots[:, 0, eb, :], k1t[:, ci, 2 * eb:2 * eb + 2, :],
                        vx, start=(gc == 0 and eb == 0), stop=last)
                    nc.tensor.matmul(
                        dots[:, 1, eb, :], k2t[:, ci, 2 * eb:2 * eb + 2, :],
                        vx, start=False, stop=last)

            for nt in range(NT):
                ns = nt * 512
                ux_t = uxp.tile([128, CC + 2, 512], bf16)
                nc.sync.dma_start(ux_t[:].rearrange("p c n -> p (c n)"), uxT[nt, :, :])
                if nt == 0:
                    # big tables land after tile-0's inputs so the first
                    # matmuls aren't stuck behind 2.7MB of table DMA
                    nc.sync.dma_start(
                        wkv_sb[:].rearrange("p c e -> p (c e)"), wkv[:])
                    nc.sync.dma_start(
                        cosP_sb[:].rearrange("p t d -> p (t d)"), cosP[:])
                    nc.sync.dma_start(
                        sinNs_sb[:].rearrange("p t d -> p (t d)"), sinNs[:])

                # ---- k/v projections; kv5 = [a(k/v), ci, g, d] ----
                kv5 = work.tile([128, 2, CPT, HPG, DH], bf16, tag="kv5")
                for ci in range(CPT):
                    kvp = kps.tile([128, 2 * E], f32, tag="kvp")
                    for cc in range(CC):
                        nc.tensor.matmul(
                            kvp[:], ux_t[:, cc, ci * 128:(ci + 1) * 128],
                            wkv_sb[:, cc, :],
                            start=(cc == 0), stop=(cc == CC - 1))
                    nc.scalar.copy(
                        kv5[:, :, ci, :, :],
                        kvp[:].rearrange("p (a g d) -> p a g d", a=2, g=HPG))

                # ---- stats: square, grouped reduce, fused sqrt(ss/64+eps) ----
                sq5 = work.tile([128, 2, CPT, HPG, DH], bf16, tag="sq5")
                nc.scalar.square(sq5[:], kv5[:])
                ss = stats.tile([128, 2, CPT, HPG], f32, tag="ss")
                nc.vector.tensor_reduce(
                    out=ss[:], in_=sq5[:], axis=mybir.AxisListType.X,
                    op=mybir.AluOpType.add)
                ts = stats.tile([128, 2, CPT * HPG], f32, tag="ts")
                nc.scalar.activation(
                    ts[:].rearrange("p a x -> p (a x)"),
                    ss[:].rearrange("p a c g -> p (a c g)"),
                    mybir.ActivationFunctionType.Sqrt,
                    bias=eps_sb[:], scale=1.0 / DH)
                sd = stats.tile([128, CPT * HPG], f32, tag="sd")
                nc.vector.tensor_mul(sd[:], ts[:, 0], ts[:, 1])
                rc = stats.tile([128, CPT, HPG], f32, tag="rc")
                nc.vector.reciprocal(rc[:].rearrange("p c g -> p (c g)"), sd[:])

                # ---- normalize k (rc folded) + rotary muls (no swap) ----
                ktn = dwork.tile([128, CPT, HPG, DH], bf16, tag="ktn")
                if nt < NT - 2:
                    nc.gpsimd.tensor_mul(
                        ktn[:], kv5[:, 0],
                        rc[:].unsqueeze(-1).broadcast_to([128, CPT, HPG, DH]))
                else:
                    nc.gpsimd.tensor_mul(
                        ktn[:, 0:2], kv5[:, 0, 0:2],
                        rc[:, 0:2].unsqueeze(-1).broadcast_to([128, 2, HPG, DH]))
                    nc.vector.tensor_mul(
                        ktn[:, 2:4], kv5[:, 0, 2:4],
                        rc[:, 2:4].unsqueeze(-1).broadcast_to([128, 2, HPG, DH]))
                k1 = dwork.tile([128, CPT, HPG, DH], bf16, tag="k1")
                k2n = dwork.tile([128, CPT, HPG, DH], bf16, tag="k2n")
                tsl = slice(nt * CPT, (nt + 1) * CPT)
                nc.vector.tensor_mul(
                    k1[:, 0:2], ktn[:, 0:2],
                    cosP_sb[:, nt * CPT:nt * CPT + 2, :].unsqueeze(2)
                    .broadcast_to([128, 2, HPG, DH]))
                nc.gpsimd.tensor_mul(
                    k1[:, 2:4], ktn[:, 2:4],
                    cosP_sb[:, nt * CPT + 2:nt * CPT + 4, :].unsqueeze(2)
                    .broadcast_to([128, 2, HPG, DH]))
                if nt < NT - 2:
                    nc.gpsimd.tensor_mul(
                        k2n[:], ktn[:],
                        sinNs_sb[:, tsl, :].unsqueeze(2).broadcast_to(
                            [128, CPT, HPG, DH]))
                else:
                    nc.gpsimd.tensor_mul(
                        k2n[:, 0:2], ktn[:, 0:2],
                        sinNs_sb[:, nt * CPT:nt * CPT + 2, :].unsqueeze(2)
                        .broadcast_to([128, 2, HPG, DH]))
                    nc.vector.tensor_mul(
                        k2n[:, 2:4], ktn[:, 2:4],
                        sinNs_sb[:, nt * CPT + 2:nt * CPT + 4, :].unsqueeze(2)
                        .broadcast_to([128, 2, HPG, DH]))

                # ---- transposed q projection + rotary premul (swap-free) ----
                for eb in range(HG):
                    qp = qps.tile([128, 512], f32, tag="qp")
                    for cc in range(CC):
                        nc.tensor.matmul(
                            qp[:], wq_sb[:, cc, eb * 128:(eb + 1) * 128],
                            ux_t[:, cc, :],
                            start=(cc == 0), stop=(cc == CC - 1))
                    nc.vector.tensor_mul(
                        t12T_sb[:, :, eb, ns:ns + 512],
                        qp[:].unsqueeze(1).broadcast_to([128, 2, 512]),
                        ux_t[:, CC:CC + 2, :])

                # ---- dots, two tiles behind (software pipeline) ----
                if nt > 1:
                    emit_dots(nt - 2)
                pend[nt] = (k1, k2n, kv5)
            emit_dots(NT - 2)
            emit_dots(NT - 1)

            # ---- finalize: dots[c] = D1[c] + G[sigma(c)] (diag blocks only),
            #      dotsS[c] = D1[sigma(c)] + G[c]; bf16, no scaling (1/N is in
            #      the q tables). sigma swaps 16-blocks within 32-blocks.
            nc.vector.memset(dotsA_sb[:], 0.0)
            # gpsimd can't read PSUM, and two-SBUF-input TensorTensor needs
            # equal partition bases -- but PSUM+SBUF with shifted bases is
            # allowed. Stage G in SBUF per eb, add on vector, then build
            # dotsS as sigma-row-block copies of the finished dotsA
            # (identical values, parallel on the scalar engine).
            gsb = stats.tile([128, HG, 128], f32, tag="gsb")
            for eb in range(HG):
                nc.scalar.copy(gsb[:, eb, :], dots[:, 1, eb, :])
                for hh in range(2):          # head-in-pair: rows/cols 64-block
                    r0, c0 = hh * 64, hh * 64
                    for b in range(2):       # 32-partition sigma blocks
                        rb = r0 + b * 32
                        rs = r0 + (b ^ 1) * 32   # sigma partner block
                        cols = slice(c0, c0 + 64)
                        nc.vector.tensor_add(
                            dotsA_sb[rb:rb + 32, eb, cols],
                            dots[rb:rb + 32, 0, eb, cols],
                            gsb[rs:rs + 32, eb, cols])
            for b in range(4):
                rb, rs = b * 32, (b ^ 1) * 32
                nc.scalar.copy(dotsS_sb[rb:rb + 32, :, :],
                               dotsA_sb[rs:rs + 32, :, :])

        # ---- pass 2: uT[vfeat, tok] = dotsA^T@t1T + dotsS^T@t2T ----
        with ExitStack() as p2:
            ups = p2.enter_context(tc.tile_pool(name="ups", bufs=6, space="PSUM"))
            uout = p2.enter_context(tc.tile_pool(name="uout", bufs=2))
            ostage = [None, None]
            for nt in range(NT):
                ns = nt * 512
                if nt % OB == 0:
                    ostage = [uout.tile([128, OB, 512], bf16, tag=f"os{eb}",
                                        name=f"os{eb}") for eb in range(HG)]
                for eb in range(HG):
                    up = ups.tile([128, 512], f32)
                    nc.tensor.matmul(up[:], dotsA_sb[:, eb, :],
                                     t12T_sb[:, 0, eb, ns:ns + 512],
                                     start=True, stop=False)
                    nc.tensor.matmul(up[:], dotsS_sb[:, eb, :],
                                     t12T_sb[:, 1, eb, ns:ns + 512],
                                     start=False, stop=True)
                    dst = ostage[eb][:, nt % OB, :]
                    if (nt + eb) % 2 == 0:
                        nc.vector.tensor_scalar(
                            out=dst, in0=up[:], scalar1=1.0, scalar2=None,
                            op0=mybir.AluOpType.mult)
                    else:
                        nc.scalar.copy(dst, up[:])
                if nt % OB == OB - 1:
                    bs = (nt // OB) * OB * 512
                    for eb in range(HG):
                        nc.sync.dma_start(
                            outT[eb, :, bs:bs + OB * 512],
                            ostage[eb][:].rearrange("p o n -> p (o n)"))

    nc.finalize()
    return nc


def _center_heads(w):
    """InstanceNorm mean-subtraction folded into weights: per 64-row head
    block, subtract the block's column means."""
    wh = w.reshape(HPG, DH, DIM)
    return (wh - wh.mean(axis=1, keepdims=True)).reshape(E, DIM)


# rotary-axis permutation: pairs (c, sigma(c)) land 32 apart, so the sigma
# reindex in the kernel is a clean 32-partition-block swap
_PI = np.concatenate([np.arange(0, 16), np.arange(32, 48),
                      np.arange(16, 32), np.arange(48, 64)])


def _permq(w):
    """Permute per-head rotary output rows of a [E, DIM] projection weight."""
    return w.reshape(HPG, DH, DIM)[:, _PI, :].reshape(E, DIM)


def _host_prep(u_x, pos_x, Wq, Wk, Wv):
    bf = ml_dtypes.bfloat16
    invf = 1.0 / 10000.0 ** (np.arange(0, 32, dtype=np.float64)[::2] / 32)
    t64 = pos_x[0].astype(np.float64) * 64.0
    fx = t64[:, 0:1] * invf[None, :]
    fy = t64[:, 1:2] * invf[None, :]
    cx, sx = np.cos(fx), np.sin(fx)
    cy, sy = np.cos(fy), np.sin(fy)
    cosPf = np.concatenate([cx, cx, cy, cy], 1).astype(np.float32)[:, _PI]
    sinNf = -np.concatenate([sx, -sx, sy, -sy], 1).astype(np.float32)[:, _PI]

    sig64 = np.arange(64) ^ 32
    sinNsf = sinNf[:, sig64]          # d-permuted for the k-side G matmuls

    def chunked(t):  # [N, 64] -> [128, NCHUNK*64], partition = token % 128
        return np.ascontiguousarray(
            t.reshape(NCHUNK, 128, DH).transpose(1, 0, 2).reshape(128, -1)).astype(bf)

    cosP = chunked(cosPf)
    sinNs = chunked(sinNsf)

    # transposed q-path tables with 1/N folded in; sin rows sigma-permuted
    cosT = np.tile(cosPf.T, (2, 1)) / N            # [128, N]
    sinTsw = np.tile(sinNsf.T, (2, 1)) / N
    rot2c = np.ascontiguousarray(
        np.stack([cosT.reshape(128, NT, 512), sinTsw.reshape(128, NT, 512)],
                 axis=2).transpose(1, 0, 2, 3).reshape(NT, 128, 1024)).astype(bf)
    rot2f = rot2c.astype(np.float32)               # appended to uxT per tile

    def wlayout(wT):  # [512, E'] -> [128, CC*E'] partition-native
        Ep = wT.shape[1]
        return np.ascontiguousarray(
            wT.reshape(CC, 128, Ep).transpose(1, 0, 2).reshape(128, -1)).astype(bf)

    in_maps = []
    for b in range(B):
        uxtb = np.ascontiguousarray(np.concatenate([
            u_x[b].reshape(NT, 512, CC, 128).transpose(0, 3, 2, 1).reshape(
                NT, 128, CC * 512),
            rot2f], axis=2)).astype(bf)
        for hg in range(HG):
            sl = slice(hg * E, (hg + 1) * E)
            wk_c = _permq(_center_heads(Wk[sl]))
            wv_c = _center_heads(Wv[sl])
            in_maps.append({
                "uxT": uxtb,
                "wq": wlayout(_permq(Wq[sl]).T.astype(np.float32)),
                "wkv": wlayout(np.concatenate([wk_c.T, wv_c.T], 1)),
                "cosP": cosP, "sinNs": sinNs,
            })
    return in_maps


def kernel(u_x, pos_x, Wq, Wk, Wv, _trace=False, _trace_dir=None):
    from concourse.bass_utils import run_bass_kernel_spmd

    if "nc" not in _cache:
        _cache["nc"] = _build_program()
    nc = _cache["nc"]

    in_maps = _host_prep(
        np.asarray(u_x, np.float32), np.asarray(pos_x, np.float32),
        np.asarray(Wq, np.float32), np.asarray(Wk, np.float32),
        np.asarray(Wv, np.float32))

    kw = {}
    if _trace:
        kw = {"trace": True, "tmpdir": _trace_dir}
    res = run_bass_kernel_spmd(nc, in_maps, core_ids=list(range(8)), **kw)
    _cache["last_result"] = res

    out = np.empty((B, N, H * DH), np.float32)
    for i in range(8):
        b, hg = divmod(i, HG)
        oT = res.results[i]["outT"].astype(np.float32)   # [HG, 128, N]
        out[b, :, hg * E:(hg + 1) * E] = oT.reshape(HG * 128, N).T
    return out
